# revision 12
# baseline (speedup 1.0000x reference)
"""Trainium2 Bass kernel for nn_CDFVarianceLoss.

Math (per sample b, per tensor z in {pred[b], target[b]}, N = 65536):
    z' = (z - min z) / (max z - min z + 1e-6)
    h_j = sum_n exp(-(z'_n - c_j)^2 / (2*sigma^2)) + 1e-6,  c_j = j/63, j < 64
    cdf = cumsum(h / sum_j h)
    loss = mean_{b,j} (cdf_pred[b,j] - cdf_target[b,j])^2

Key identity: the cumulative kernel sums S_j = sum_n V_j(z'_n) with
V_j(z) = sum_{k<=j} exp(-alpha (z-c_k)^2) are smooth sigmoid-like functions
of z.  Each V_j is approximated (offline least squares; rel error on the
loss ~4e-4 against a 2e-2 gate) by a linear combination of M=6 shifted
erfs plus a constant:
    V_j(z) ~= sum_k R[j,k] erf((g_k - z)/s2) + R[j,M]
so the device only computes the M basis sums D_k = sum_n erf((g_k-z')/s2)
per array -- M activation passes over the natural [128, 512] layout with
scalar scale/bias (the erf argument is affine in z) -- and the host
applies R in float64.  This replaces the O(N*BINS) exp/matmul grids of
the direct approach with O(N*M) activation work.

Distribution: data-parallel over the batch -- 16 samples over 8 cores,
2 samples per core (4 arrays of 65536 per core: pred/target x 2 samples).

Per-core pipeline:
  - 8 half-array loads spread over the three DMA-capable queues; each
    array's (min, max) DVE reduces start as soon as its two pieces land
  - one gpsimd cross-lane max finishes (-zmin, zmax) for all arrays;
    fixups on partition 0; PE ones-matmul broadcast; DVE normalize
  - M x ACT Erf passes [128, 2048] (all 4 arrays at once), fp16 outputs
  - per-pass DVE chase: fp16 pair-fold (2x mode) + segmented tensor_reduce
    -> fp32 per-partition partials red[:, k*4+a]
  - red [128, 4M] goes to DRAM as-is; the host does the 128-way partial
    sum, applies R, and computes the exact eps/normalize/cumsum/mse tail
    in float64
"""

import math

import numpy as np

B = 16
N = 65536
BINS = 64
SIGMA = 0.05
EPS = 1e-6
ALPHA = 0.5 / SIGMA**2  # 200.0
NCORES = 8
SPC = B // NCORES  # samples per core
NARR = 2 * SPC  # arrays per core: (pred, target) x samples
P = 128
F = N // P  # 512 natural free dim

# erf basis (designed offline; see module docstring)
M = 4
G_LO, G_HI = -0.04, 1.12
SIG_FIT = 0.20
S2 = SIG_FIT * math.sqrt(2.0)
G_PTS = [G_LO + (G_HI - G_LO) * k / (M - 1) for k in range(M)]

_CACHE = {}


def _fit_R():
    """Least-squares fit of V_j(z) in the erf basis + constant (fp64)."""
    nz = 40001
    zg = np.linspace(0.0, 1.0, nz)
    c = np.linspace(0.0, 1.0, BINS)
    K = np.exp(-ALPHA * (zg[None, :] - c[:, None]) ** 2)
    Vt = np.cumsum(K, axis=0)  # [64, nz]
    g = np.asarray(G_PTS)
    erf_v = np.vectorize(math.erf)
    Fb = np.concatenate(
        [erf_v((g[:, None] - zg[None, :]) / S2), np.ones((1, nz))], axis=0
    )  # [M+1, nz]
    w = np.ones(nz)
    w[zg < 0.02] = 3.0
    w[zg > 0.98] = 3.0
    Aw = Fb * w[None, :]
    Gm = Aw @ Fb.T
    Rhs = (Vt * w[None, :]) @ Fb.T
    return np.linalg.solve(Gm + 1e-10 * np.eye(M + 1), Rhs.T).T  # [64, M+1]


def _build_nc():
    import concourse.bass as bass  # noqa: F401
    import concourse.bacc as bacc
    import concourse.tile as tile
    from concourse import mybir
    from contextlib import ExitStack

    f32 = mybir.dt.float32
    f16 = mybir.dt.float16
    AX = mybir.AxisListType
    OP = mybir.AluOpType
    ACTF = mybir.ActivationFunctionType

    nc = bacc.Bacc()
    pred_d = nc.declare_dram_parameter("pred", [SPC, N], f32, isOutput=False)
    targ_d = nc.declare_dram_parameter("target", [SPC, N], f32, isOutput=False)
    out_d = nc.declare_dram_parameter("dsums", [P, M * NARR], f32, isOutput=True)

    ones_row_np = np.ones((1, P), np.float32)
    bias_np = np.broadcast_to(
        np.asarray([g / S2 for g in G_PTS], np.float32)[None, :], (P, M)
    ).copy()

    ones_d = nc.inline_tensor(ones_row_np, name="ones_row")
    bias_d = nc.inline_tensor(bias_np, name="erf_bias")

    with tile.TileContext(nc) as tc, ExitStack() as ctx:
        singles = ctx.enter_context(tc.tile_pool(name="singles", bufs=1))
        nat = ctx.enter_context(tc.tile_pool(name="nat", bufs=1))
        eo_pool = ctx.enter_context(tc.tile_pool(name="eo", bufs=2))
        fold_pool = ctx.enter_context(tc.tile_pool(name="fold", bufs=2))
        small = ctx.enter_context(tc.tile_pool(name="small", bufs=2))
        ps_pool = ctx.enter_context(tc.tile_pool(name="ps", bufs=2, space="PSUM"))

        # natural input, arrays side by side on the free dim: [128, 4*512];
        # two half-loads per array spread over the DMA-capable queues so
        # each array's min/max can start as soon as its pieces land
        zn = nat.tile([P, NARR * F], f32, name="zn")
        srcs = [pred_d[0, :], pred_d[1, :], targ_d[0, :], targ_d[1, :]]
        qs = [nc.sync, nc.gpsimd, nc.scalar]
        H = F // 2
        qi = 0
        for a, src in enumerate(srcs):
            s2d = src.rearrange("(p f) -> p f", p=P)
            for piece in range(2):
                qs[qi % 3].dma_start(
                    out=zn[:, a * F + piece * H : a * F + (piece + 1) * H],
                    in_=s2d[:, piece * H : (piece + 1) * H],
                )
                qi += 1

        ones_sb = singles.tile([1, P], f32)
        nc.sync.dma_start(out=ones_sb, in_=ones_d[:, :])
        bias_sb = singles.tile([P, M], f32)
        nc.scalar.dma_start(out=bias_sb, in_=bias_d[:, :])

        # per-array (-min, max), one reduce pair per tensor (pred cols load
        # first, so its reduces overlap the target tensor's transfers)
        mm = small.tile([P, 2 * NARR], f32, tag="mm")
        for t in range(2):
            sl3 = zn[:, t * 2 * F : (t + 1) * 2 * F].rearrange(
                "p (a f) -> p a f", f=F
            )
            nc.vector.tensor_reduce(
                out=mm[:, 2 * t : 2 * t + 2], in_=sl3, axis=AX.X,
                op=OP.min, negate=True,
            )
            nc.vector.tensor_reduce(
                out=mm[:, NARR + 2 * t : NARR + 2 * t + 2], in_=sl3, axis=AX.X,
                op=OP.max,
            )
        # cross-partition finish on gpsimd: [1, 8] = (-zmin x4 | zmax x4)
        mn_all = small.tile([1, 2 * NARR], f32, tag="mn")
        nc.gpsimd.tensor_reduce(out=mn_all, in_=mm, axis=AX.C, op=OP.max)

        # fixups on partition 0: nbsrc = [-zmin_a x4 | 1/(zmax-zmin+eps) x4]
        nbsrc = small.tile([1, 2 * NARR], f32, tag="nbsrc")
        rng = small.tile([1, NARR], f32, tag="rng")
        nc.vector.tensor_tensor(
            out=rng,
            in0=mn_all[0:1, 0:NARR],
            in1=mn_all[0:1, NARR : 2 * NARR],
            op=OP.add,
        )
        nc.vector.tensor_scalar_add(rng, rng, EPS)
        nc.vector.reciprocal(nbsrc[0:1, NARR : 2 * NARR], rng)
        nc.vector.tensor_copy(nbsrc[0:1, 0:NARR], mn_all[0:1, 0:NARR])

        # broadcast to all partitions with a ones-column matmul
        nbp = ps_pool.tile([P, 2 * NARR], f32, tag="ps")
        nc.tensor.matmul(nbp, ones_sb[:, :], nbsrc, start=True, stop=True)
        nb = small.tile([P, 2 * NARR], f32, tag="nb")
        nc.vector.tensor_copy(nb, nbp)

        # normalize each array -> z' in [0, 1]
        zc = nat.tile([P, NARR * F], f32, name="zc")
        for a in range(NARR):
            nc.vector.tensor_scalar(
                zc[:, a * F : (a + 1) * F],
                zn[:, a * F : (a + 1) * F],
                nb[:, a : a + 1],
                nb[:, NARR + a : NARR + a + 1],
                OP.add,
                OP.mult,
            )

        # erf spine; per-pass fp16 pair-fold (2x mode) + segmented reduce
        red = small.tile([P, M * NARR], f32, tag="red", name="red")
        HF = F // 2
        for k in range(M):
            eo = eo_pool.tile([P, NARR * F], f16, tag="eo")
            nc.scalar.activation(
                out=eo,
                in_=zc,
                func=ACTF.Erf,
                bias=bias_sb[:, k : k + 1],
                scale=float(-1.0 / S2),
            )
            eo3 = eo.rearrange("p (a f) -> p a f", f=F)
            fold = fold_pool.tile([P, NARR * HF], f16, tag="fold")
            fold3 = fold.rearrange("p (a h) -> p a h", h=HF)
            nc.vector.tensor_tensor(
                out=fold3, in0=eo3[:, :, 0:HF], in1=eo3[:, :, HF:F], op=OP.add
            )
            nc.vector.tensor_reduce(
                out=red[:, k * NARR : (k + 1) * NARR],
                in_=fold3,
                axis=AX.X,
                op=OP.add,
            )

        # ship the per-partition partials; the host does the 128-way sum
        nc.scalar.dma_start(out=out_d[:, :], in_=red)

    nc.compile()
    return nc


def kernel(pred: np.ndarray, target: np.ndarray) -> np.ndarray:
    from concourse.bass_utils import run_bass_kernel_spmd

    if "nc" not in _CACHE:
        _CACHE["nc"] = _build_nc()
        _CACHE["R"] = _fit_R()
    nc = _CACHE["nc"]
    R = _CACHE["R"]

    pred = np.ascontiguousarray(np.asarray(pred, np.float32).reshape(B, N))
    target = np.ascontiguousarray(np.asarray(target, np.float32).reshape(B, N))
    in_maps = [
        {
            "pred": pred[i * SPC : (i + 1) * SPC],
            "target": target[i * SPC : (i + 1) * SPC],
        }
        for i in range(NCORES)
    ]
    res = run_bass_kernel_spmd(nc, in_maps, list(range(NCORES)))

    # device dsums layout [128, M*NARR]; col k*NARR + a with a in
    # (pred_s0, targ_s0, pred_s1, targ_s1); sum over partitions on host
    Dp = np.zeros((B, M + 1))
    Dt = np.zeros((B, M + 1))
    Dp[:, M] = N
    Dt[:, M] = N
    for core in range(NCORES):
        raw = np.asarray(res.results[core]["dsums"], np.float64).reshape(P, M, NARR)
        sums = raw.sum(axis=0)  # [M, NARR]
        for s in range(SPC):
            b = core * SPC + s
            Dp[b, :M] = sums[:, s]
            Dt[b, :M] = sums[:, SPC + s]

    Sx = Dp @ R.T  # [B, 64] cumulative kernel sums
    Sy = Dt @ R.T
    js = np.arange(1, BINS + 1, dtype=np.float64)
    cdf_x = (Sx + js[None, :] * EPS) / (Sx[:, -1:] + BINS * EPS)
    cdf_y = (Sy + js[None, :] * EPS) / (Sy[:, -1:] + BINS * EPS)
    return np.float32(np.mean((cdf_x - cdf_y) ** 2))


# revision 13
# speedup vs baseline: 1.2201x; 1.2201x over previous
"""Trainium2 Bass kernel for nn_CDFVarianceLoss.

Math (per sample b, per tensor z in {pred[b], target[b]}, N = 65536):
    z' = (z - min z) / (max z - min z + 1e-6)
    h_j = sum_n exp(-(z'_n - c_j)^2 / (2*sigma^2)) + 1e-6,  c_j = j/63, j < 64
    cdf = cumsum(h / sum_j h)
    loss = mean_{b,j} (cdf_pred[b,j] - cdf_target[b,j])^2

Key identity: the cumulative kernel sums S_j = sum_n V_j(z'_n) with
V_j(z) = sum_{k<=j} exp(-alpha (z-c_k)^2) are smooth sigmoid-like functions
of z.  Each V_j is approximated (offline least squares; rel error on the
loss ~4e-4 against a 2e-2 gate) by a linear combination of M=6 shifted
erfs plus a constant:
    V_j(z) ~= sum_k R[j,k] erf((g_k - z)/s2) + R[j,M]
so the device only computes the M basis sums D_k = sum_n erf((g_k-z')/s2)
per array -- M activation passes over the natural [128, 512] layout with
scalar scale/bias (the erf argument is affine in z) -- and the host
applies R in float64.  This replaces the O(N*BINS) exp/matmul grids of
the direct approach with O(N*M) activation work.

Distribution: data-parallel over the batch -- 16 samples over 8 cores,
2 samples per core (4 arrays of 65536 per core: pred/target x 2 samples).

Per-core pipeline:
  - 8 half-array loads spread over the three DMA-capable queues; each
    array's (min, max) DVE reduces start as soon as its two pieces land
  - one gpsimd cross-lane max finishes (-zmin, zmax) for all arrays;
    fixups on partition 0; PE ones-matmul broadcast; DVE normalize
  - M x ACT Erf passes [128, 2048] (all 4 arrays at once), fp16 outputs
  - per-pass DVE chase: fp16 pair-fold (2x mode) + segmented tensor_reduce
    -> fp32 per-partition partials red[:, k*4+a]
  - red [128, 4M] goes to DRAM as-is; the host does the 128-way partial
    sum, applies R, and computes the exact eps/normalize/cumsum/mse tail
    in float64
"""

import math

import numpy as np

B = 16
N = 65536
BINS = 64
SIGMA = 0.05
EPS = 1e-6
ALPHA = 0.5 / SIGMA**2  # 200.0
NCORES = 8
SPC = B // NCORES  # samples per core
NARR = 2 * SPC  # arrays per core: (pred, target) x samples
P = 128
F = N // P  # 512 natural free dim

# erf basis (designed offline; see module docstring)
M = 4
G_LO, G_HI = -0.04, 1.12
SIG_FIT = 0.20
S2 = SIG_FIT * math.sqrt(2.0)
G_PTS = [G_LO + (G_HI - G_LO) * k / (M - 1) for k in range(M)]

_CACHE = {}


def _fit_R():
    """Least-squares fit of V_j(z) in the erf basis + constant (fp64)."""
    nz = 40001
    zg = np.linspace(0.0, 1.0, nz)
    c = np.linspace(0.0, 1.0, BINS)
    K = np.exp(-ALPHA * (zg[None, :] - c[:, None]) ** 2)
    Vt = np.cumsum(K, axis=0)  # [64, nz]
    g = np.asarray(G_PTS)
    erf_v = np.vectorize(math.erf)
    Fb = np.concatenate(
        [erf_v((g[:, None] - zg[None, :]) / S2), np.ones((1, nz))], axis=0
    )  # [M+1, nz]
    w = np.ones(nz)
    w[zg < 0.02] = 3.0
    w[zg > 0.98] = 3.0
    Aw = Fb * w[None, :]
    Gm = Aw @ Fb.T
    Rhs = (Vt * w[None, :]) @ Fb.T
    return np.linalg.solve(Gm + 1e-10 * np.eye(M + 1), Rhs.T).T  # [64, M+1]


def _build_nc():
    import concourse.bass as bass  # noqa: F401
    import concourse.bacc as bacc
    import concourse.tile as tile
    from concourse import mybir
    from contextlib import ExitStack

    f32 = mybir.dt.float32
    f16 = mybir.dt.float16
    AX = mybir.AxisListType
    OP = mybir.AluOpType
    ACTF = mybir.ActivationFunctionType

    nc = bacc.Bacc()
    pred_d = nc.declare_dram_parameter("pred", [SPC, N], f32, isOutput=False)
    targ_d = nc.declare_dram_parameter("target", [SPC, N], f32, isOutput=False)
    out_d = nc.declare_dram_parameter("dsums", [P, M * NARR], f32, isOutput=True)

    ones_row_np = np.ones((1, P), np.float32)
    bias_np = np.broadcast_to(
        np.asarray([g / S2 for g in G_PTS], np.float32)[None, :], (P, M)
    ).copy()

    ones_d = nc.inline_tensor(ones_row_np, name="ones_row")
    bias_d = nc.inline_tensor(bias_np, name="erf_bias")

    with tile.TileContext(nc) as tc, ExitStack() as ctx:
        singles = ctx.enter_context(tc.tile_pool(name="singles", bufs=1))
        nat = ctx.enter_context(tc.tile_pool(name="nat", bufs=1))
        eo_pool = ctx.enter_context(tc.tile_pool(name="eo", bufs=2))
        fold_pool = ctx.enter_context(tc.tile_pool(name="fold", bufs=2))
        small = ctx.enter_context(tc.tile_pool(name="small", bufs=2))
        ps_pool = ctx.enter_context(tc.tile_pool(name="ps", bufs=2, space="PSUM"))

        # natural input, arrays side by side on the free dim: [128, 4*512];
        # two half-loads per array spread over the DMA-capable queues so
        # each array's min/max can start as soon as its pieces land
        zn = nat.tile([P, NARR * F], f32, name="zn")
        srcs = [pred_d[0, :], pred_d[1, :], targ_d[0, :], targ_d[1, :]]
        qs = [nc.sync, nc.gpsimd, nc.scalar]
        H = F // 2
        qi = 0
        for a, src in enumerate(srcs):
            s2d = src.rearrange("(p f) -> p f", p=P)
            for piece in range(2):
                qs[qi % 3].dma_start(
                    out=zn[:, a * F + piece * H : a * F + (piece + 1) * H],
                    in_=s2d[:, piece * H : (piece + 1) * H],
                )
                qi += 1

        ones_sb = singles.tile([1, P], f32)
        nc.sync.dma_start(out=ones_sb, in_=ones_d[:, :])
        bias_sb = singles.tile([P, M], f32)
        nc.scalar.dma_start(out=bias_sb, in_=bias_d[:, :])

        # per-array (-min, max), one reduce pair per tensor (pred cols load
        # first, so its reduces overlap the target tensor's transfers)
        mm = small.tile([P, 2 * NARR], f32, tag="mm")
        for t in range(2):
            sl3 = zn[:, t * 2 * F : (t + 1) * 2 * F].rearrange(
                "p (a f) -> p a f", f=F
            )
            nc.vector.tensor_reduce(
                out=mm[:, 2 * t : 2 * t + 2], in_=sl3, axis=AX.X,
                op=OP.min, negate=True,
            )
            nc.vector.tensor_reduce(
                out=mm[:, NARR + 2 * t : NARR + 2 * t + 2], in_=sl3, axis=AX.X,
                op=OP.max,
            )
        # cross-partition finish on gpsimd: [1, 8] = (-zmin x4 | zmax x4)
        mn_all = small.tile([1, 2 * NARR], f32, tag="mn")
        nc.gpsimd.tensor_reduce(out=mn_all, in_=mm, axis=AX.C, op=OP.max)

        # fixups on partition 0: nbsrc = [-zmin_a x4 | 1/(zmax-zmin+eps) x4]
        nbsrc = small.tile([1, 2 * NARR], f32, tag="nbsrc")
        rng = small.tile([1, NARR], f32, tag="rng")
        nc.vector.tensor_tensor(
            out=rng,
            in0=mn_all[0:1, 0:NARR],
            in1=mn_all[0:1, NARR : 2 * NARR],
            op=OP.add,
        )
        nc.vector.tensor_scalar_add(rng, rng, EPS)
        nc.vector.reciprocal(nbsrc[0:1, NARR : 2 * NARR], rng)
        nc.vector.tensor_copy(nbsrc[0:1, 0:NARR], mn_all[0:1, 0:NARR])

        # broadcast to all partitions with a ones-column matmul
        nbp = ps_pool.tile([P, 2 * NARR], f32, tag="ps")
        nc.tensor.matmul(nbp, ones_sb[:, :], nbsrc, start=True, stop=True)
        nb = small.tile([P, 2 * NARR], f32, tag="nb")
        nc.vector.tensor_copy(nb, nbp)

        # normalize each array -> z' in [0, 1]
        zc = nat.tile([P, NARR * F], f32, name="zc")
        for a in range(NARR):
            nc.vector.tensor_scalar(
                zc[:, a * F : (a + 1) * F],
                zn[:, a * F : (a + 1) * F],
                nb[:, a : a + 1],
                nb[:, NARR + a : NARR + a + 1],
                OP.add,
                OP.mult,
            )

        # erf spine; per-pass fp16 pair-fold (2x mode) + segmented reduce
        red = small.tile([P, M * NARR], f32, tag="red", name="red")
        HF = F // 2
        TW = 2 * F  # per-tensor width
        for k in range(M):
            eo = eo_pool.tile([P, NARR * F], f16, tag="eo")
            if k == 0:
                # split the first pass per tensor: the pred half only waits
                # for pred's normalizes, starting the spine earlier
                for t in range(2):
                    nc.scalar.activation(
                        out=eo[:, t * TW : (t + 1) * TW],
                        in_=zc[:, t * TW : (t + 1) * TW],
                        func=ACTF.Erf,
                        bias=bias_sb[:, k : k + 1],
                        scale=float(-1.0 / S2),
                    )
            else:
                nc.scalar.activation(
                    out=eo,
                    in_=zc,
                    func=ACTF.Erf,
                    bias=bias_sb[:, k : k + 1],
                    scale=float(-1.0 / S2),
                )
            eo3 = eo.rearrange("p (a f) -> p a f", f=F)
            fold = fold_pool.tile([P, NARR * HF], f16, tag="fold")
            fold3 = fold.rearrange("p (a h) -> p a h", h=HF)
            nc.vector.tensor_tensor(
                out=fold3, in0=eo3[:, :, 0:HF], in1=eo3[:, :, HF:F], op=OP.add
            )
            nc.vector.tensor_reduce(
                out=red[:, k * NARR : (k + 1) * NARR],
                in_=fold3,
                axis=AX.X,
                op=OP.add,
            )

        # ship the per-partition partials; the host does the 128-way sum
        nc.scalar.dma_start(out=out_d[:, :], in_=red)

    nc.compile()
    return nc


def kernel(pred: np.ndarray, target: np.ndarray) -> np.ndarray:
    from concourse.bass_utils import run_bass_kernel_spmd

    if "nc" not in _CACHE:
        _CACHE["nc"] = _build_nc()
        _CACHE["R"] = _fit_R()
    nc = _CACHE["nc"]
    R = _CACHE["R"]

    pred = np.ascontiguousarray(np.asarray(pred, np.float32).reshape(B, N))
    target = np.ascontiguousarray(np.asarray(target, np.float32).reshape(B, N))
    in_maps = [
        {
            "pred": pred[i * SPC : (i + 1) * SPC],
            "target": target[i * SPC : (i + 1) * SPC],
        }
        for i in range(NCORES)
    ]
    res = run_bass_kernel_spmd(nc, in_maps, list(range(NCORES)))

    # device dsums layout [128, M*NARR]; col k*NARR + a with a in
    # (pred_s0, targ_s0, pred_s1, targ_s1); sum over partitions on host
    Dp = np.zeros((B, M + 1))
    Dt = np.zeros((B, M + 1))
    Dp[:, M] = N
    Dt[:, M] = N
    for core in range(NCORES):
        raw = np.asarray(res.results[core]["dsums"], np.float64).reshape(P, M, NARR)
        sums = raw.sum(axis=0)  # [M, NARR]
        for s in range(SPC):
            b = core * SPC + s
            Dp[b, :M] = sums[:, s]
            Dt[b, :M] = sums[:, SPC + s]

    Sx = Dp @ R.T  # [B, 64] cumulative kernel sums
    Sy = Dt @ R.T
    js = np.arange(1, BINS + 1, dtype=np.float64)
    cdf_x = (Sx + js[None, :] * EPS) / (Sx[:, -1:] + BINS * EPS)
    cdf_y = (Sy + js[None, :] * EPS) / (Sy[:, -1:] + BINS * EPS)
    return np.float32(np.mean((cdf_x - cdf_y) ** 2))
